# revision 2
# baseline (speedup 1.0000x reference)
"""HL1 ACE loss kernel for Trainium2, 8-core data-parallel over spatial.

Per core: softmax over C=4 on its spatial shard, then cumulative threshold
families computed with fused compare+reduce instructions:
  C_k  = #{p_c >= t_k}            (counts, per (b,c))
  A_k  = sum relu(p_c - t_k)      (for per-bin prob sums, per (b,c))
  U_j  = #{u >= u_j},  u = p_tgt + label   (target-class counts, per b)
Host decodes per-bin histograms from the cumulative sums (summed over
8 cores and 128 partitions) and computes the final scalar.
"""
import sys
sys.path.insert(0, "/opt/trn_rl_repo")
import numpy as np

B, C = 4, 4
NBINS = 15
NCORES = 8
SP_FULL = 128 * 128 * 128          # spatial per (b,c), full
SP = SP_FULL // NCORES             # spatial per core = 262144
P, F = 128, SP // 128              # sbuf tile geometry 128 x 2048

EPS32 = np.float32(np.finfo(np.float32).eps)
BOUNDS = np.linspace(np.float32(0.0), np.float32(1.0) + EPS32, NBINS + 1,
                     dtype=np.float32)          # [16] matches reference
TK = BOUNDS[1:]                # thresholds t_1..t_15 (t_15 unused)


def _build(nc_mod, bass, mybir, f_override=None):
    """Emit the SPMD program. Returns (nc, colmaps) where colmaps describe
    accum column meanings for host decode."""
    Fl = f_override or F
    f32 = mybir.dt.float32
    i32 = mybir.dt.int32
    nc = nc_mod

    lg = nc.dram_tensor("lg", [B, C, P, Fl], f32, kind="ExternalInput")
    lb = nc.dram_tensor("lb", [B, P, Fl], i32, kind="ExternalInput")

    # ---- accum column bookkeeping -------------------------------------
    # families: ('C', b, c, k) k=1..14 | ('A', b, c, k) k=0..14 | ('U', b, j) j=1..59
    ACT_PLANES = (0, 1, 2)
    work = []
    for b in range(B):
        for c in range(C):
            for k in range(1, 15):
                work.append(("C", b, c, k))
            if c in ACT_PLANES:
                for k in range(0, 15):
                    work.append(("A", b, c, k))
            else:
                for k in range(1, 16):
                    work.append(("S", b, c, k))   # CS_k = sum min(p, t_k)
        for j in range(1, 60):
            work.append(("U", b, j))

    # engine split: ACT takes A-family (relu accum is native) for planes c<3,
    # DVE takes C for c<2 + A for c=3 + all fixed work, GP takes the rest.
    def eng_of(item):
        fam = item[0]
        if fam == "A":
            return "A"
        if fam in ("C", "S"):
            return "V"
        return "A"  # U family on ACT via Sign

    cols = {"V": [], "A": [], "G": []}
    for it in work:
        cols[eng_of(it)].append(it)
    colmap = {e: {it: i for i, it in enumerate(cols[e])} for e in cols}

    accs_shape = {e: max(1, len(cols[e])) for e in cols}
    outV = nc.dram_tensor("outV", [P, accs_shape["V"]], f32, kind="ExternalOutput")
    outA = nc.dram_tensor("outA", [P, accs_shape["A"]], f32, kind="ExternalOutput")
    outG = nc.dram_tensor("outG", [P, accs_shape["G"]], f32, kind="ExternalOutput")

    # ---- const bias APs for ACT (negated thresholds) -------------------
    # U threshold value for j: u_j = (j // 15) + TK[(j % 15) - 1]... handle below

    def u_thresh(j):
        c, k = divmod(j, 15)
        if k == 0:
            return float(c)  # boundary exactly at integer c
        return float(c) + float(TK[k - 1])

    bias_vals = set()
    for it in cols["A"]:
        if it[0] == "A" and it[3] >= 1:
            bias_vals.add(-float(TK[it[3] - 1]))
        elif it[0] == "C":
            bias_vals.add(-float(TK[it[3] - 1]))
        elif it[0] == "U":
            bias_vals.add(-u_thresh(it[2]))
    for v in sorted(bias_vals):
        t = nc.alloc_sbuf_tensor(f"cb_{abs(v):.7f}".replace(".", "_"), [P, 1], f32)
        nc.gpsimd.memset(t.ap(), v)
        nc.const_aps.aps[(f32, v)] = t.ap()
    nc.all_engine_barrier()

    # ---- sbuf tiles ----------------------------------------------------
    def sb(name, shape, dt=f32):
        return nc.alloc_sbuf_tensor(name, shape, dt).ap()

    lgs = [sb(f"lgs{i}", [P, C * Fl]) for i in range(2)]
    lbs = [sb(f"lbs{i}", [P, Fl], i32) for i in range(2)]
    e = sb("e", [P, C * Fl])                     # exp / q in-place
    ssum = sb("ssum", [P, Fl])
    rcp = sb("rcp", [P, Fl])
    lbf = sb("lbf", [P, Fl])
    qtg = sb("qtg", [P, Fl])
    tmp = sb("tmp", [P, Fl])
    u = sb("u", [P, Fl])
    scrV = sb("scrV", [P, Fl])
    scrA = sb("scrA", [P, Fl])
    scrG = sb("scrG", [P, Fl])
    accV = sb("accV", [P, accs_shape["V"]])
    accA = sb("accA", [P, accs_shape["A"]])
    accG = sb("accG", [P, accs_shape["G"]])

    AL = mybir.AluOpType
    AF = mybir.ActivationFunctionType

    def pview(buf, c):
        return buf[:, c * Fl:(c + 1) * Fl]

    with (
        nc.Block() as block,
        nc.semaphore("dma_sem") as dma_sem,
        nc.semaphore("a_sem") as a_sem,
        nc.semaphore("v_sem") as v_sem,
        nc.semaphore("g_sem") as g_sem,
    ):
        @block.sync
        def _(sync):
            for b in range(B):
                if b >= 2:
                    sync.wait_ge(v_sem, 3 * (b - 2) + 3)
                    sync.wait_ge(a_sem, 2 * (b - 2) + 2)
                    sync.wait_ge(g_sem, (b - 2) + 1)
                buf = b % 2
                sync.dma_start(out=lgs[buf], in_=lg[b]).then_inc(dma_sem, 16)
                sync.dma_start(out=lbs[buf], in_=lb[b]).then_inc(dma_sem, 16)
            sync.wait_ge(v_sem, 3 * B)
            sync.wait_ge(a_sem, 2 * B)
            sync.wait_ge(g_sem, B)
            sync.dma_start(out=outV[:], in_=accV).then_inc(dma_sem, 16)
            sync.dma_start(out=outA[:], in_=accA).then_inc(dma_sem, 16)
            sync.dma_start(out=outG[:], in_=accG).then_inc(dma_sem, 16)
            sync.wait_ge(dma_sem, 32 * B + 48)

        @block.scalar
        def _(act):
            for b in range(B):
                buf = b % 2
                if b >= 1:
                    act.wait_ge(v_sem, 3 * b)       # DVE done reading e of b-1
                act.wait_ge(dma_sem, 32 * b + 16)   # lg DMA of b done
                for c in range(C):
                    ins = act.activation(out=pview(e, c), in_=pview(lgs[buf], c),
                                         func=AF.Exp)
                ins.then_inc(a_sem, 1)              # exps done: a=2b+1
                act.wait_ge(v_sem, 3 * b + 1)       # q ready
                items = [it for it in cols["A"] if it[1] == b]
                items.sort(key=lambda it: it[0] == "U")
                waited_u = False
                for it in items:
                    fam = it[0]
                    if fam == "U" and not waited_u:
                        act.wait_ge(v_sem, 3 * b + 2)
                        waited_u = True
                    ci = colmap["A"][it]
                    ao = accA[:, ci:ci + 1]
                    if fam == "A":
                        k = it[3]
                        bias = 0.0 if k == 0 else -float(TK[k - 1])
                        ins = act.activation(out=scrA, in_=pview(e, it[2]),
                                             func=AF.Relu, bias=bias, accum_out=ao)
                    elif fam == "C":
                        ins = act.activation(out=scrA, in_=pview(e, it[2]),
                                             func=AF.Sign,
                                             bias=-float(TK[it[3] - 1]), accum_out=ao)
                    else:  # U
                        ins = act.activation(out=scrA, in_=u, func=AF.Sign,
                                             bias=-u_thresh(it[2]), accum_out=ao)
                ins.then_inc(a_sem, 1)              # a = 2b+2

        @block.vector
        def _(vec):
            for b in range(B):
                buf = b % 2
                vec.wait_ge(a_sem, 2 * b + 1)       # exps of b done
                vec.tensor_add(ssum, pview(e, 0), pview(e, 1))
                vec.tensor_add(ssum, ssum, pview(e, 2))
                vec.tensor_add(ssum, ssum, pview(e, 3))
                vec.reciprocal(out=rcp, in_=ssum)
                for c in range(C):                  # q_c = e_c * rcp (in place)
                    ins = vec.tensor_mul(pview(e, c), pview(e, c), rcp)
                ins.then_inc(v_sem, 1)              # v = 3b+1 : q ready
                vec.tensor_copy(out=lbf, in_=lbs[buf])
                vec.scalar_tensor_tensor(out=qtg, in0=lbf, scalar=0.0,
                                         in1=pview(e, 0), op0=AL.is_equal, op1=AL.mult)
                for c in range(1, C):
                    vec.scalar_tensor_tensor(out=tmp, in0=lbf, scalar=float(c),
                                             in1=pview(e, c), op0=AL.is_equal,
                                             op1=AL.mult)
                    vec.tensor_add(qtg, qtg, tmp)
                ins = vec.scalar_tensor_tensor(out=u, in0=lbf, scalar=1.0, in1=qtg,
                                               op0=AL.mult, op1=AL.add)
                ins.then_inc(v_sem, 1)              # v = 3b+2 : u ready
                ins = None
                for it in cols["V"]:
                    if it[1] != b:
                        continue
                    fam = it[0]
                    ci = colmap["V"][it]
                    ao = accV[:, ci:ci + 1]
                    if fam == "C":
                        ins = vec.tensor_scalar(out=scrV, in0=pview(e, it[2]),
                                                scalar1=float(TK[it[3] - 1]),
                                                scalar2=0.0, op0=AL.is_ge,
                                                op1=AL.add, accum_out=ao)
                    elif fam == "S":
                        ins = vec.tensor_scalar(out=scrV, in0=pview(e, it[2]),
                                                scalar1=float(TK[it[3] - 1]),
                                                scalar2=0.0, op0=AL.min,
                                                op1=AL.add, accum_out=ao)
                    else:
                        ins = vec.tensor_scalar(out=scrV, in0=u,
                                                scalar1=u_thresh(it[2]),
                                                scalar2=0.0, op0=AL.is_ge,
                                                op1=AL.add, accum_out=ao)
                ins.then_inc(v_sem, 1)              # v = 3b+3 : done with b

        @block.gpsimd
        def _(gp):
            for b in range(B):
                gp.sem_inc(g_sem, 1)

    return nc, colmap, u_thresh


def _decode(colmap, res_list, n_spatial_bc, n_pos_b):
    """Host decode: res_list = per-core dicts outV/outA/outG [128, n]."""
    sums = {}
    for e, key in (("V", "outV"), ("A", "outA"), ("G", "outG")):
        tot = np.zeros(res_list[0][key].shape[1], np.float64)
        for r in res_list:
            tot += r[key].astype(np.float64).sum(0)
        for it, ci in colmap[e].items():
            sums[it] = tot[ci]


    Cf = np.zeros((B, C, 16))            # C_k cumulative counts, k=0..15
    Af = np.zeros((B, C, 16))            # A_k relu moments
    Uf = np.zeros((B, 61))               # U_j cumulative counts j=0..60
    Cf[:, :, 0] = n_spatial_bc
    Uf[:, 0] = n_pos_b
    N = n_spatial_bc
    CS = np.full((B, C, 17), np.nan)
    for it, v in sums.items():
        if it[0] == "C":
            Cf[it[1], it[2], it[3]] = v
        elif it[0] == "A":
            Af[it[1], it[2], it[3]] = v
        elif it[0] == "S":
            CS[it[1], it[2], it[3]] = v
        else:
            Uf[it[1], it[2]] = v
    for b in range(B):
        for c in range(C):
            if not np.isnan(CS[b, c, 15]):
                Af[b, c, 0] = CS[b, c, 15]
                for k in range(1, 15):
                    Af[b, c, k] = CS[b, c, 15] - CS[b, c, k]
    return Cf, Af, Uf


def _finalize(Cf, Af, Uf, act_sign_items):
    # fix sign-encoded counts: C = (v + N)/2 for items computed via ACT Sign
    for (fam, b, c, k), n in act_sign_items:
        if fam == "C":
            Cf[b, c, k] = (Cf[b, c, k] + n) / 2.0
    tk = np.zeros(16)
    tk[0] = 0.0
    tk[1:15] = TK[:14].astype(np.float64)
    tk[15] = float(BOUNDS[15])

    cnt = Cf[:, :, :15] - Cf[:, :, 1:16]               # counts per bin
    S = Af[:, :, :15] + tk[:15] * Cf[:, :, :15]        # S_k = A_k + t_k C_k
    Sb = np.zeros((B, C, 15))
    Sb[:, :, :14] = S[:, :, :14] - S[:, :, 1:15]
    Sb[:, :, 14] = S[:, :, 14]
    # target counts from U
    TC = np.zeros((B, C, 16))
    for c in range(C):
        for k in range(15):
            TC[:, c, k] = Uf[:, 15 * c + k] - (Uf[:, 15 * (c + 1)] if c < 3 else 0.0)
    tcb = TC[:, :, :15].copy()
    tcb[:, :, :14] = TC[:, :, :14] - TC[:, :, 1:15]
    tcb[:, :, 14] = TC[:, :, 14]
    Tc = TC[:, :, 0]                                   # total label==c count

    valid = cnt > 0.5
    denom = np.where(valid, cnt, 1.0)
    mean_p = Sb / denom
    mean_t = tcb / denom
    diff = np.where(valid, np.abs(mean_p - mean_t), 0.0)
    n_valid = np.maximum(valid.sum(-1), 1)
    ace = diff.sum(-1) / n_valid
    non_empty = (Tc > 0.5).astype(np.float64)
    return np.float32((ace * non_empty).mean())


def kernel(logits, labels):
    import concourse.bass as bass
    from concourse import mybir
    from concourse.bass_utils import run_bass_kernel_spmd

    nc = bass.Bass()
    nc, colmap, _ = _build(nc, bass, mybir)

    lgf = np.ascontiguousarray(logits.reshape(B, C, SP_FULL), np.float32)
    lbl = np.ascontiguousarray(labels.reshape(B, SP_FULL)).astype(np.int32)

    in_maps = []
    for i in range(NCORES):
        sl = slice(i * SP, (i + 1) * SP)
        in_maps.append({
            "lg": np.ascontiguousarray(lgf[:, :, sl]).reshape(B, C, P, F),
            "lb": np.ascontiguousarray(lbl[:, sl]).reshape(B, P, F),
        })
    import os
    trace = bool(int(os.environ.get("KERNEL_TRACE", "0")))
    tmpdir = os.environ.get("KERNEL_TMPDIR") or None
    res = run_bass_kernel_spmd(nc, in_maps, list(range(NCORES)), trace=trace,
                               tmpdir=tmpdir)
    results = res.results

    Cf, Af, Uf = _decode(colmap, results, SP_FULL, SP_FULL)
    # ACT 'C'/'U' entries are sign-encoded
    act_items = [(it, SP_FULL) for it in colmap["A"] if it[0] == "C"]
    for it in colmap["A"]:
        if it[0] == "U":
            Uf[it[1], it[2]] = (Uf[it[1], it[2]] + SP_FULL) / 2.0
    out = _finalize(Cf, Af, Uf, act_items)
    kernel._last = res
    return out



# revision 4
# speedup vs baseline: 1.6925x; 1.6925x over previous
"""HL1 ACE loss kernel for Trainium2, 8-core data-parallel over spatial.

Per core (per batch b): softmax over C=4 on the spatial shard, then the
three per-(b,c,bin) histogram families via cumulative thresholds:
  C_k = #{p_c >= t_k}          counts
  A_k = sum relu(p_c - t_k)    prob-mass above t_k  (=> per-bin sum_p)
  T_k = #{lab==c & p_c >= t_k} target counts
Custom DVE micro-ops pack TWO counts per pass into one f32 accumulator
(lo + 4096*hi; both fields <= 2048 so the sum stays integer-exact under
2^24), and fuse p-materialization with A_0 (MULSUM) and mask-build with
T_0 (MASKSUM). ACT carries exp, the relu (A) singles and a few sign (C)
singles. Host decodes the tiny [128, ncols] accumulators.
"""
import sys
sys.path.insert(0, "/opt/trn_rl_repo")
import os
import numpy as np

B, C = 4, 4
NBINS = 15
NCORES = 8
SP_FULL = 128 * 128 * 128          # spatial per (b,c), full problem
SP = SP_FULL // NCORES             # spatial per core = 262144
P, F = 128, SP // 128              # sbuf tile geometry 128 x 2048

EPS32 = np.float32(np.finfo(np.float32).eps)
BOUNDS = np.linspace(np.float32(0.0), np.float32(1.0) + EPS32, NBINS + 1,
                     dtype=np.float32)
TK = BOUNDS[1:]                    # t_1..t_15 (t_15 = 1+eps, never used)

PK = 4096.0                        # packing field multiplier

# ACT takes these C-thresholds as Sign singles; the rest pair up on DVE.
SIGN_C = [(0, 14), (1, 14), (2, 14), (3, 14),
          (0, 13), (1, 13), (2, 13), (3, 13)]


# ---- custom DVE op registration ------------------------------------------
def _register_ops():
    import concourse.dve_ops as dops
    from concourse.dve_spec import (Spec, Src0, Src1, C0, C1, C2, relu, eq,
                                    lower, _has_src1)
    from concourse.dve_uop import DveOpSpec
    from operator import add as _add

    def reg(name, body, accum=None, reference=None):
        for o in dops.OPS:
            if o.name == name:
                return o
        row = dops._CUSTOM_DVE_ROW_BASE + len(dops.OPS)
        spec = Spec(body=body, accum=accum, reference=reference)
        sha = {}
        for ver in ("v3", "v4"):
            u = lower(spec, ver=ver)
            sha[ver] = DveOpSpec(name=name, opcode=row, uops=u,
                                 rd1_en=_has_src1(spec)).sha(ver)
        op = dops.DveOp(name, spec, subdim=False, uops_sha=sha)
        dops.OPS.append(op)
        dops._SUB_OPCODE_FOR_NAME[name] = row
        dops.CUSTOM_DVE_SPECS[name] = spec
        return op

    cpack = reg("CPACK_K", (Src0 >= C0) + C2 * (Src0 >= C1), accum=_add,
                reference=lambda in0, s0, s1, imm2:
                (in0 >= s0) + imm2 * (in0 >= s1))
    tpack = reg("TPACK_K", ((Src0 >= C0) + C2 * (Src0 >= C1)) * Src1,
                accum=_add,
                reference=lambda in0, in1, s0, s1, imm2:
                ((in0 >= s0) + imm2 * (in0 >= s1)) * in1)
    mulsum = reg("MULSUM_K", Src0 * Src1, accum=_add,
                 reference=lambda in0, in1, s0, s1, imm2: in0 * in1)
    masksum = reg("MASKSUM_K", eq(Src0, C0), accum=_add,
                  reference=lambda in0, s0, s1, imm2:
                  (in0 == s0).astype(np.float32))
    return cpack, tpack, mulsum, masksum


def _build(nc, mybir):
    """Emit the SPMD program. Returns (nc, dve_cols, act_cols)."""
    CPACK, TPACK, MULSUM, MASKSUM = _register_ops()
    f32 = mybir.dt.float32
    AF = mybir.ActivationFunctionType
    AL = mybir.AluOpType

    lg = nc.dram_tensor("lg", [B, C, P, F], f32, kind="ExternalInput")
    lb = nc.dram_tensor("lb", [B, P, F], f32, kind="ExternalInput")

    # ---- column bookkeeping ------------------------------------------
    # DVE: ("A0",b,c) | ("T0",b,c) | ("CC",b,c,klo,khi) | ("TP",b,c,klo,khi)
    # ACT: ("A",b,c,k) k=1..14 | ("CS",b,c,k) for SIGN_C
    dve_cols, act_cols = [], []
    sign_c = {}
    for (c, k) in SIGN_C:
        sign_c.setdefault(c, set()).add(k)
    for b in range(B):
        for c in range(C):
            dve_cols.append(("A0", b, c))
            dve_cols.append(("T0", b, c))
            cks = [k for k in range(1, 15) if k not in sign_c.get(c, ())]
            if len(cks) % 2:
                cks.append(cks[-1])
            for i in range(0, len(cks), 2):
                dve_cols.append(("CC", b, c, cks[i], cks[i + 1]))
            tks = list(range(1, 15)) + [14]    # 14 thr -> 7 pairs (pad dup)
            for i in range(0, 14, 2):
                dve_cols.append(("TP", b, c, tks[i], tks[i + 1]))
            for k in range(1, 15):
                act_cols.append(("A", b, c, k))
            for k in sorted(sign_c.get(c, ())):
                act_cols.append(("CS", b, c, k))
    dmap = {it: i for i, it in enumerate(dve_cols)}
    amap = {it: i for i, it in enumerate(act_cols)}

    outV = nc.dram_tensor("outV", [P, len(dve_cols)], f32,
                          kind="ExternalOutput")
    outA = nc.dram_tensor("outA", [P, len(act_cols)], f32,
                          kind="ExternalOutput")

    # ---- const bias APs for ACT --------------------------------------
    bias_vals = {0.0}
    for k in range(1, 15):
        bias_vals.add(-float(TK[k - 1]))
    for v in sorted(bias_vals):
        t = nc.alloc_sbuf_tensor(
            f"cb_{abs(v):.7f}".replace(".", "_") + ("m" if v < 0 else "p"),
            [P, 1], f32)
        nc.gpsimd.memset(t.ap(), v)
        nc.const_aps.aps[(f32, v)] = t.ap()
    nc.all_engine_barrier()

    # ---- sbuf tiles ---------------------------------------------------
    def sb(name, shape, dt=f32):
        return nc.alloc_sbuf_tensor(name, shape, dt).ap()

    lgs = [sb(f"lgs{i}", [P, C * F]) for i in range(2)]   # logits -> e (exp)
    lbs = sb("lbs", [P, F])                               # labels (f32)
    ps = [sb(f"ps{i}", [P, C * F]) for i in range(2)]     # softmax probs
    S = sb("S", [P, F])
    R = sb("R", [P, F])
    rscr = sb("rscr", [P, F])
    m = sb("m", [P, F])                                    # per-class mask
    scr = sb("scr", [P, F])                                # packed-op out
    ascr = sb("ascr", [P, F])                              # ACT singles out
    accV = sb("accV", [P, len(dve_cols)])
    accA = sb("accA", [P, len(act_cols)])

    def pview(buf, c):
        return buf[:, c * F:(c + 1) * F]

    with (
        nc.Block() as block,
        nc.semaphore("dma_sem") as dma_sem,
        nc.semaphore("ae_sem") as ae_sem,      # ACT exp(b) done: b+1
        nc.semaphore("as_sem") as as_sem,      # ACT singles(b) done: b+1
        nc.semaphore("vp_sem") as vp_sem,      # DVE p(b) ready: b+1
        nc.semaphore("vd_sem") as vd_sem,      # DVE packed(b) done: b+1
    ):
        @block.sync
        def _(sync):
            for b in range(B):
                if b >= 2:
                    sync.wait_ge(vd_sem, b - 1)
                sync.dma_start(out=lgs[b % 2], in_=lg[b]).then_inc(dma_sem, 16)
                if b >= 1:
                    sync.wait_ge(vd_sem, b)
                sync.dma_start(out=lbs, in_=lb[b]).then_inc(dma_sem, 16)
            sync.wait_ge(vd_sem, B)
            sync.wait_ge(as_sem, B)
            sync.dma_start(out=outV[:], in_=accV).then_inc(dma_sem, 16)
            sync.dma_start(out=outA[:], in_=accA).then_inc(dma_sem, 16)
            sync.wait_ge(dma_sem, 32 * B + 32)

        @block.scalar
        def _(act):
            def exp(b):
                act.wait_ge(dma_sem, 32 * b + 16)
                for c in range(C):
                    ins = act.activation(out=pview(lgs[b % 2], c),
                                         in_=pview(lgs[b % 2], c), func=AF.Exp)
                ins.then_inc(ae_sem, 1)

            def singles(b):
                act.wait_ge(vp_sem, b + 1)
                pb = ps[b % 2]
                ins = None
                for (fam, bb, c, k) in act_cols:
                    if bb != b:
                        continue
                    ao = accA[:, amap[(fam, bb, c, k)]:amap[(fam, bb, c, k)] + 1]
                    func = AF.Relu if fam == "A" else AF.Sign
                    ins = act.activation(out=ascr, in_=pview(pb, c),
                                         func=func, bias=-float(TK[k - 1]),
                                         accum_out=ao)
                ins.then_inc(as_sem, 1)

            exp(0)
            exp(1)
            singles(0)
            exp(2)
            singles(1)
            exp(3)
            singles(2)
            singles(3)

        @block.vector
        def _(vec):
            for b in range(B):
                buf = b % 2
                e = lgs[buf]
                pb = ps[buf]
                vec.wait_ge(ae_sem, b + 1)
                vec.tensor_add(S, pview(e, 0), pview(e, 1))
                vec.tensor_add(S, S, pview(e, 2))
                vec.tensor_add(S, S, pview(e, 3))
                vec.reciprocal_approx_accurate(out=R, in_=S, scratch=rscr)
                if b >= 2:
                    vec.wait_ge(as_sem, b - 1)
                for c in range(C):
                    ao = accV[:, dmap[("A0", b, c)]:dmap[("A0", b, c)] + 1]
                    ins = vec._custom_dve(MULSUM, out=pview(pb, c),
                                          in0=pview(e, c), in1=R,
                                          accum_out=ao)
                ins.then_inc(vp_sem, 1)
                vec.wait_ge(dma_sem, 32 * b + 32)
                for c in range(C):
                    ao = accV[:, dmap[("T0", b, c)]:dmap[("T0", b, c)] + 1]
                    vec._custom_dve(MASKSUM, out=m, in0=lbs,
                                    s0=float(c), accum_out=ao)
                    for it in dve_cols:
                        if it[0] == "TP" and it[1] == b and it[2] == c:
                            _, _, _, klo, khi = it
                            ao2 = accV[:, dmap[it]:dmap[it] + 1]
                            vec._custom_dve(
                                TPACK, out=scr, in0=pview(pb, c), in1=m,
                                s0=float(TK[klo - 1]), s1=float(TK[khi - 1]),
                                imm2=PK, accum_out=ao2)
                    for it in dve_cols:
                        if it[0] == "CC" and it[1] == b and it[2] == c:
                            _, _, _, klo, khi = it
                            ao2 = accV[:, dmap[it]:dmap[it] + 1]
                            ins = vec._custom_dve(
                                CPACK, out=scr, in0=pview(pb, c),
                                s0=float(TK[klo - 1]), s1=float(TK[khi - 1]),
                                imm2=PK, accum_out=ao2)
                ins.then_inc(vd_sem, 1)

    return nc, dve_cols, act_cols, dmap, amap


def _decode(dve_cols, act_cols, results):
    """Sum per-core [128, n] accumulators and decode into the cumulative
    family arrays Cf[b,c,k], Af[b,c,k], Tf[b,c,k] (k = 0..15)."""
    NV = len(dve_cols)
    totV = np.zeros(NV, np.float64)
    totA = np.zeros(len(act_cols), np.float64)
    # packed columns must be decoded per partition-row per core (fields are
    # only guaranteed <= 2048 per row), so split lo/hi before summing.
    lo_acc = np.zeros(NV, np.float64)
    hi_acc = np.zeros(NV, np.float64)
    for r in results:
        v = r["outV"].astype(np.float64)        # [128, NV]
        hi = np.floor(v / PK)
        lo = v - hi * PK
        lo_acc += lo.sum(0)
        hi_acc += hi.sum(0)
        totV += v.sum(0)
        totA += r["outA"].astype(np.float64).sum(0)

    Cf = np.zeros((B, C, 16))
    Af = np.zeros((B, C, 16))
    Tf = np.zeros((B, C, 16))
    Cf[:, :, 0] = SP_FULL
    n_cores = len(results)
    for i, it in enumerate(dve_cols):
        fam = it[0]
        if fam == "A0":
            Af[it[1], it[2], 0] = totV[i]
        elif fam == "T0":
            Tf[it[1], it[2], 0] = totV[i]
        elif fam == "CC":
            _, b, c, klo, khi = it
            Cf[b, c, klo] = lo_acc[i]
            Cf[b, c, khi] = hi_acc[i]
        else:  # TP
            _, b, c, klo, khi = it
            Tf[b, c, klo] = lo_acc[i]
            Tf[b, c, khi] = hi_acc[i]
    for i, it in enumerate(act_cols):
        fam, b, c, k = it
        if fam == "A":
            Af[b, c, k] = totA[i]
        else:  # CS: sign-encoded count
            Cf[b, c, k] = (totA[i] + SP_FULL) / 2.0
    return Cf, Af, Tf


def _finalize(Cf, Af, Tf):
    tk = np.zeros(16)
    tk[1:16] = TK.astype(np.float64)
    cnt = Cf[:, :, :15] - Cf[:, :, 1:16]
    S = Af[:, :, :15] + tk[:15] * Cf[:, :, :15]
    Sb = np.zeros((B, C, 15))
    Sb[:, :, :14] = S[:, :, :14] - S[:, :, 1:15]
    Sb[:, :, 14] = S[:, :, 14]
    tcb = Tf[:, :, :15] - Tf[:, :, 1:16]

    valid = cnt > 0.5
    denom = np.where(valid, cnt, 1.0)
    mean_p = Sb / denom
    mean_t = tcb / denom
    diff = np.where(valid, np.abs(mean_p - mean_t), 0.0)
    n_valid = np.maximum(valid.sum(-1), 1)
    ace = diff.sum(-1) / n_valid
    non_empty = (Tf[:, :, 0] > 0.5).astype(np.float64)
    return np.float32((ace * non_empty).mean())


def kernel(logits, labels):
    import concourse.bass as bass
    from concourse import mybir
    from concourse.bass_utils import run_bass_kernel_spmd

    nc = bass.Bass()
    nc, dve_cols, act_cols, dmap, amap = _build(nc, mybir)
    mybir.codegen_inst_isa_subclasses(nc)   # encode custom-DVE ISA bytes

    lgf = np.ascontiguousarray(np.asarray(logits).reshape(B, C, SP_FULL),
                               np.float32)
    lbl = np.asarray(labels).reshape(B, SP_FULL).astype(np.float32)

    in_maps = []
    for i in range(NCORES):
        sl = slice(i * SP, (i + 1) * SP)
        in_maps.append({
            "lg": np.ascontiguousarray(lgf[:, :, sl]).reshape(B, C, P, F),
            "lb": np.ascontiguousarray(lbl[:, sl]).reshape(B, P, F),
        })
    trace = bool(int(os.environ.get("KERNEL_TRACE", "0")))
    tmpdir = os.environ.get("KERNEL_TMPDIR") or None
    res = run_bass_kernel_spmd(nc, in_maps, list(range(NCORES)), trace=trace,
                               tmpdir=tmpdir)
    Cf, Af, Tf = _decode(dve_cols, act_cols, res.results)
    out = _finalize(Cf, Af, Tf)
    kernel._last = res
    return out


# revision 6
# speedup vs baseline: 1.7344x; 1.0248x over previous
"""HL1 ACE loss kernel for Trainium2, 8-core data-parallel over spatial.

Per core (per batch b): softmax over C=4 on the spatial shard, then the
three per-(b,c,bin) histogram families via cumulative thresholds:
  C_k = #{p_c >= t_k}          counts
  A_k = sum relu(p_c - t_k)    prob-mass above t_k  (=> per-bin sum_p)
  T_k = #{lab==c & p_c >= t_k} target counts
Custom DVE micro-ops pack TWO counts per pass into one f32 accumulator
(lo + 4096*hi; both fields <= 2048 so the sum stays integer-exact under
2^24), and fuse p-materialization with A_0 (MULSUM) and mask-build with
T_0 (MASKSUM). ACT carries exp, the relu (A) singles and a few sign (C)
singles. Host decodes the tiny [128, ncols] accumulators.
"""
import sys
sys.path.insert(0, "/opt/trn_rl_repo")
import os
import numpy as np

B, C = 4, 4
NBINS = 15
NCORES = 8
SP_FULL = 128 * 128 * 128          # spatial per (b,c), full problem
SP = SP_FULL // NCORES             # spatial per core = 262144
P, F = 128, SP // 128              # sbuf tile geometry 128 x 2048

EPS32 = np.float32(np.finfo(np.float32).eps)
BOUNDS = np.linspace(np.float32(0.0), np.float32(1.0) + EPS32, NBINS + 1,
                     dtype=np.float32)
TK = BOUNDS[1:]                    # t_1..t_15 (t_15 = 1+eps, never used)

PK = 4096.0                        # packing field multiplier

# ACT takes these C-thresholds as Sign singles; the rest pair up on DVE.
SIGN_C = [(0, 14), (1, 14), (2, 14), (3, 14),
          (0, 13), (1, 13), (2, 13), (3, 13)]


# ---- custom DVE op registration ------------------------------------------
def _register_ops():
    import concourse.dve_ops as dops
    from concourse.dve_spec import (Spec, Src0, Src1, C0, C1, C2, relu, eq,
                                    lower, _has_src1)
    from concourse.dve_uop import DveOpSpec
    from operator import add as _add

    def reg(name, body, accum=None, reference=None):
        for o in dops.OPS:
            if o.name == name:
                return o
        row = dops._CUSTOM_DVE_ROW_BASE + len(dops.OPS)
        spec = Spec(body=body, accum=accum, reference=reference)
        sha = {}
        for ver in ("v3", "v4"):
            u = lower(spec, ver=ver)
            sha[ver] = DveOpSpec(name=name, opcode=row, uops=u,
                                 rd1_en=_has_src1(spec)).sha(ver)
        op = dops.DveOp(name, spec, subdim=False, uops_sha=sha)
        dops.OPS.append(op)
        dops._SUB_OPCODE_FOR_NAME[name] = row
        dops.CUSTOM_DVE_SPECS[name] = spec
        return op

    cpack = reg("CPACK_K", (Src0 >= C0) + C2 * (Src0 >= C1), accum=_add,
                reference=lambda in0, s0, s1, imm2:
                (in0 >= s0) + imm2 * (in0 >= s1))
    tpack = reg("TPACK_K", ((Src0 >= C0) + C2 * (Src0 >= C1)) * Src1,
                accum=_add,
                reference=lambda in0, in1, s0, s1, imm2:
                ((in0 >= s0) + imm2 * (in0 >= s1)) * in1)
    mulsum = reg("MULSUM_K", Src0 * Src1, accum=_add,
                 reference=lambda in0, in1, s0, s1, imm2: in0 * in1)
    masksum = reg("MASKSUM_K", eq(Src0, C0), accum=_add,
                  reference=lambda in0, s0, s1, imm2:
                  (in0 == s0).astype(np.float32))
    return cpack, tpack, mulsum, masksum


def _build(nc, mybir):
    """Emit the SPMD program. Returns (nc, dve_cols, act_cols)."""
    CPACK, TPACK, MULSUM, MASKSUM = _register_ops()
    f32 = mybir.dt.float32
    AF = mybir.ActivationFunctionType
    AL = mybir.AluOpType

    lg = nc.dram_tensor("lg", [B, C, P, F], f32, kind="ExternalInput")
    lb = nc.dram_tensor("lb", [B, P, F], f32, kind="ExternalInput")

    # ---- column bookkeeping ------------------------------------------
    # DVE: ("A0",b,c) | ("T0",b,c) | ("CC",b,c,klo,khi) | ("TP",b,c,klo,khi)
    # ACT: ("A",b,c,k) k=1..14 | ("CS",b,c,k) for SIGN_C
    dve_cols, act_cols = [], []
    sign_c = {}
    for (c, k) in SIGN_C:
        sign_c.setdefault(c, set()).add(k)
    for b in range(B):
        for c in range(C):
            dve_cols.append(("A0", b, c))
            dve_cols.append(("T0", b, c))
            cks = [k for k in range(1, 15) if k not in sign_c.get(c, ())]
            if len(cks) % 2:
                cks.append(cks[-1])
            for i in range(0, len(cks), 2):
                dve_cols.append(("CC", b, c, cks[i], cks[i + 1]))
            tks = list(range(1, 15)) + [14]    # 14 thr -> 7 pairs (pad dup)
            for i in range(0, 14, 2):
                dve_cols.append(("TP", b, c, tks[i], tks[i + 1]))
            for k in range(1, 15):
                act_cols.append(("A", b, c, k))
            for k in sorted(sign_c.get(c, ())):
                act_cols.append(("CS", b, c, k))
    dmap = {it: i for i, it in enumerate(dve_cols)}
    amap = {it: i for i, it in enumerate(act_cols)}

    outV = nc.dram_tensor("outV", [P, len(dve_cols)], f32,
                          kind="ExternalOutput")
    outA = nc.dram_tensor("outA", [P, len(act_cols)], f32,
                          kind="ExternalOutput")

    # ---- const bias APs for ACT --------------------------------------
    bias_vals = {0.0}
    for k in range(1, 15):
        bias_vals.add(-float(TK[k - 1]))
    for v in sorted(bias_vals):
        t = nc.alloc_sbuf_tensor(
            f"cb_{abs(v):.7f}".replace(".", "_") + ("m" if v < 0 else "p"),
            [P, 1], f32)
        nc.gpsimd.memset(t.ap(), v)
        nc.const_aps.aps[(f32, v)] = t.ap()
    nc.all_engine_barrier()

    # ---- sbuf tiles ---------------------------------------------------
    def sb(name, shape, dt=f32):
        return nc.alloc_sbuf_tensor(name, shape, dt).ap()

    lgs = [sb(f"lgs{i}", [P, C * F]) for i in range(2)]   # logits -> e (exp)
    lbs = sb("lbs", [P, F])                               # labels (f32)
    ps = [sb(f"ps{i}", [P, C * F]) for i in range(2)]     # softmax probs
    S = sb("S", [P, F])
    R = sb("R", [P, F])
    rscr = sb("rscr", [P, F])
    m = sb("m", [P, F])                                    # per-class mask
    scr = sb("scr", [P, F])                                # packed-op out
    ascr = sb("ascr", [P, F])                              # ACT singles out
    accV = sb("accV", [P, len(dve_cols)])
    accA = sb("accA", [P, len(act_cols)])

    def pview(buf, c):
        return buf[:, c * F:(c + 1) * F]

    with (
        nc.Block() as block,
        nc.semaphore("dma_sem") as dma_sem,
        nc.semaphore("lg_sem") as lg_sem,
        nc.semaphore("lb_sem") as lb_sem,
        nc.semaphore("ae_sem") as ae_sem,      # ACT exp(b) done: b+1
        nc.semaphore("as_sem") as as_sem,      # ACT singles(b) done: b+1
        nc.semaphore("vp_sem") as vp_sem,      # DVE p(b) ready: b+1
        nc.semaphore("vd_sem") as vd_sem,      # DVE packed(b) done: b+1
    ):
        @block.sync
        def _(sync):
            for b in range(B):
                if b >= 2:
                    sync.wait_ge(vd_sem, b - 1)
                for c in range(C):
                    sync.dma_start(out=lgs[b % 2][:, c * F:(c + 1) * F],
                                   in_=lg[b, c]).then_inc(lg_sem, 16)
                if b >= 1:
                    sync.wait_ge(vd_sem, b)
                sync.dma_start(out=lbs, in_=lb[b]).then_inc(lb_sem, 16)
            sync.wait_ge(vd_sem, B)
            sync.wait_ge(as_sem, B)
            sync.dma_start(out=outV[:], in_=accV).then_inc(dma_sem, 16)
            sync.dma_start(out=outA[:], in_=accA).then_inc(dma_sem, 16)
            sync.wait_ge(lg_sem, 64 * B)
            sync.wait_ge(lb_sem, 16 * B)
            sync.wait_ge(dma_sem, 32)

        @block.scalar
        def _(act):
            def exp(b):
                for c in range(C):
                    act.wait_ge(lg_sem, 64 * b + 16 * (c + 1))
                    ins = act.activation(out=pview(lgs[b % 2], c),
                                         in_=pview(lgs[b % 2], c), func=AF.Exp)
                    ins.then_inc(ae_sem, 1)

            def singles(b):
                pb = ps[b % 2]
                ins = None
                for cc in range(C):
                    act.wait_ge(vp_sem, 4 * b + cc + 1)
                    for (fam, bb, c, k) in act_cols:
                        if bb != b or c != cc:
                            continue
                        i0 = amap[(fam, bb, c, k)]
                        ins = act.activation(out=ascr, in_=pview(pb, c),
                                             func=AF.Relu if fam == "A"
                                             else AF.Sign,
                                             bias=-float(TK[k - 1]),
                                             accum_out=accA[:, i0:i0 + 1])
                ins.then_inc(as_sem, 1)

            exp(0)
            exp(1)
            singles(0)
            exp(2)
            singles(1)
            exp(3)
            singles(2)
            singles(3)

        @block.vector
        def _(vec):
            for b in range(B):
                buf = b % 2
                e = lgs[buf]
                pb = ps[buf]
                vec.wait_ge(ae_sem, 4 * b + 2)
                vec.tensor_add(S, pview(e, 0), pview(e, 1))
                vec.wait_ge(ae_sem, 4 * b + 3)
                vec.tensor_add(S, S, pview(e, 2))
                vec.wait_ge(ae_sem, 4 * b + 4)
                vec.tensor_add(S, S, pview(e, 3))
                vec.reciprocal_approx_accurate(out=R, in_=S, scratch=rscr)
                if b >= 2:
                    vec.wait_ge(as_sem, b - 1)
                for c in range(C):
                    ao = accV[:, dmap[("A0", b, c)]:dmap[("A0", b, c)] + 1]
                    vec._custom_dve(MULSUM, out=pview(pb, c),
                                    in0=pview(e, c), in1=R,
                                    accum_out=ao).then_inc(vp_sem, 1)
                vec.wait_ge(lb_sem, 16 * (b + 1))
                for c in range(C):
                    ao = accV[:, dmap[("T0", b, c)]:dmap[("T0", b, c)] + 1]
                    vec._custom_dve(MASKSUM, out=m, in0=lbs,
                                    s0=float(c), accum_out=ao)
                    for it in dve_cols:
                        if it[0] == "TP" and it[1] == b and it[2] == c:
                            _, _, _, klo, khi = it
                            ao2 = accV[:, dmap[it]:dmap[it] + 1]
                            vec._custom_dve(
                                TPACK, out=scr, in0=pview(pb, c), in1=m,
                                s0=float(TK[klo - 1]), s1=float(TK[khi - 1]),
                                imm2=PK, accum_out=ao2)
                    for it in dve_cols:
                        if it[0] == "CC" and it[1] == b and it[2] == c:
                            _, _, _, klo, khi = it
                            ao2 = accV[:, dmap[it]:dmap[it] + 1]
                            ins = vec._custom_dve(
                                CPACK, out=scr, in0=pview(pb, c),
                                s0=float(TK[klo - 1]), s1=float(TK[khi - 1]),
                                imm2=PK, accum_out=ao2)
                ins.then_inc(vd_sem, 1)

    return nc, dve_cols, act_cols, dmap, amap


def _decode(dve_cols, act_cols, results):
    """Sum per-core [128, n] accumulators and decode into the cumulative
    family arrays Cf[b,c,k], Af[b,c,k], Tf[b,c,k] (k = 0..15)."""
    NV = len(dve_cols)
    totV = np.zeros(NV, np.float64)
    totA = np.zeros(len(act_cols), np.float64)
    # packed columns must be decoded per partition-row per core (fields are
    # only guaranteed <= 2048 per row), so split lo/hi before summing.
    lo_acc = np.zeros(NV, np.float64)
    hi_acc = np.zeros(NV, np.float64)
    for r in results:
        v = r["outV"].astype(np.float64)        # [128, NV]
        hi = np.floor(v / PK)
        lo = v - hi * PK
        lo_acc += lo.sum(0)
        hi_acc += hi.sum(0)
        totV += v.sum(0)
        totA += r["outA"].astype(np.float64).sum(0)

    Cf = np.zeros((B, C, 16))
    Af = np.zeros((B, C, 16))
    Tf = np.zeros((B, C, 16))
    Cf[:, :, 0] = SP_FULL
    n_cores = len(results)
    for i, it in enumerate(dve_cols):
        fam = it[0]
        if fam == "A0":
            Af[it[1], it[2], 0] = totV[i]
        elif fam == "T0":
            Tf[it[1], it[2], 0] = totV[i]
        elif fam == "CC":
            _, b, c, klo, khi = it
            Cf[b, c, klo] = lo_acc[i]
            Cf[b, c, khi] = hi_acc[i]
        else:  # TP
            _, b, c, klo, khi = it
            Tf[b, c, klo] = lo_acc[i]
            Tf[b, c, khi] = hi_acc[i]
    for i, it in enumerate(act_cols):
        fam, b, c, k = it
        if fam == "A":
            Af[b, c, k] = totA[i]
        else:  # CS: sign-encoded count
            Cf[b, c, k] = (totA[i] + SP_FULL) / 2.0
    return Cf, Af, Tf


def _finalize(Cf, Af, Tf):
    tk = np.zeros(16)
    tk[1:16] = TK.astype(np.float64)
    cnt = Cf[:, :, :15] - Cf[:, :, 1:16]
    S = Af[:, :, :15] + tk[:15] * Cf[:, :, :15]
    Sb = np.zeros((B, C, 15))
    Sb[:, :, :14] = S[:, :, :14] - S[:, :, 1:15]
    Sb[:, :, 14] = S[:, :, 14]
    tcb = Tf[:, :, :15] - Tf[:, :, 1:16]

    valid = cnt > 0.5
    denom = np.where(valid, cnt, 1.0)
    mean_p = Sb / denom
    mean_t = tcb / denom
    diff = np.where(valid, np.abs(mean_p - mean_t), 0.0)
    n_valid = np.maximum(valid.sum(-1), 1)
    ace = diff.sum(-1) / n_valid
    non_empty = (Tf[:, :, 0] > 0.5).astype(np.float64)
    return np.float32((ace * non_empty).mean())


def kernel(logits, labels):
    import concourse.bass as bass
    from concourse import mybir
    from concourse.bass_utils import run_bass_kernel_spmd

    nc = bass.Bass()
    nc, dve_cols, act_cols, dmap, amap = _build(nc, mybir)
    mybir.codegen_inst_isa_subclasses(nc)   # encode custom-DVE ISA bytes

    lgf = np.ascontiguousarray(np.asarray(logits).reshape(B, C, SP_FULL),
                               np.float32)
    lbl = np.asarray(labels).reshape(B, SP_FULL).astype(np.float32)

    in_maps = []
    for i in range(NCORES):
        sl = slice(i * SP, (i + 1) * SP)
        in_maps.append({
            "lg": np.ascontiguousarray(lgf[:, :, sl]).reshape(B, C, P, F),
            "lb": np.ascontiguousarray(lbl[:, sl]).reshape(B, P, F),
        })
    trace = bool(int(os.environ.get("KERNEL_TRACE", "0")))
    tmpdir = os.environ.get("KERNEL_TMPDIR") or None
    res = run_bass_kernel_spmd(nc, in_maps, list(range(NCORES)), trace=trace,
                               tmpdir=tmpdir)
    Cf, Af, Tf = _decode(dve_cols, act_cols, res.results)
    out = _finalize(Cf, Af, Tf)
    kernel._last = res
    return out


# revision 7
# speedup vs baseline: 1.7374x; 1.0017x over previous
"""HL1 ACE loss kernel for Trainium2, 8-core data-parallel over spatial.

Per core (per batch b): softmax over C=4 on the spatial shard, then the
three per-(b,c,bin) histogram families via cumulative thresholds:
  C_k = #{p_c >= t_k}          counts
  A_k = sum relu(p_c - t_k)    prob-mass above t_k  (=> per-bin sum_p)
  T_k = #{lab==c & p_c >= t_k} target counts
Custom DVE micro-ops pack TWO counts per pass into one f32 accumulator
(lo + 4096*hi; both fields <= 2048 so the sum stays integer-exact under
2^24), and fuse p-materialization with A_0 (MULSUM) and mask-build with
T_0 (MASKSUM). ACT carries exp, the relu (A) singles and a few sign (C)
singles. Host decodes the tiny [128, ncols] accumulators.
"""
import sys
sys.path.insert(0, "/opt/trn_rl_repo")
import os
import numpy as np

B, C = 4, 4
NBINS = 15
NCORES = 8
SP_FULL = 128 * 128 * 128          # spatial per (b,c), full problem
SP = SP_FULL // NCORES             # spatial per core = 262144
P, F = 128, SP // 128              # sbuf tile geometry 128 x 2048

EPS32 = np.float32(np.finfo(np.float32).eps)
BOUNDS = np.linspace(np.float32(0.0), np.float32(1.0) + EPS32, NBINS + 1,
                     dtype=np.float32)
TK = BOUNDS[1:]                    # t_1..t_15 (t_15 = 1+eps, never used)

PK = 4096.0                        # packing field multiplier

# ACT takes these C-thresholds as Sign singles; the rest pair up on DVE.
SIGN_C = [(0, 14), (1, 14), (2, 14), (3, 14),
          (0, 13), (1, 13), (2, 13), (3, 13)]


# ---- custom DVE op registration ------------------------------------------
def _register_ops():
    import concourse.dve_ops as dops
    from concourse.dve_spec import (Spec, Src0, Src1, C0, C1, C2, relu, eq,
                                    lower, _has_src1)
    from concourse.dve_uop import DveOpSpec
    from operator import add as _add

    def reg(name, body, accum=None, reference=None):
        for o in dops.OPS:
            if o.name == name:
                return o
        row = dops._CUSTOM_DVE_ROW_BASE + len(dops.OPS)
        spec = Spec(body=body, accum=accum, reference=reference)
        sha = {}
        for ver in ("v3", "v4"):
            u = lower(spec, ver=ver)
            sha[ver] = DveOpSpec(name=name, opcode=row, uops=u,
                                 rd1_en=_has_src1(spec)).sha(ver)
        op = dops.DveOp(name, spec, subdim=False, uops_sha=sha)
        dops.OPS.append(op)
        dops._SUB_OPCODE_FOR_NAME[name] = row
        dops.CUSTOM_DVE_SPECS[name] = spec
        return op

    cpack = reg("CPACK_K", (Src0 >= C0) + C2 * (Src0 >= C1), accum=_add,
                reference=lambda in0, s0, s1, imm2:
                (in0 >= s0) + imm2 * (in0 >= s1))
    tpack = reg("TPACK_K", ((Src0 >= C0) + C2 * (Src0 >= C1)) * Src1,
                accum=_add,
                reference=lambda in0, in1, s0, s1, imm2:
                ((in0 >= s0) + imm2 * (in0 >= s1)) * in1)
    mulsum = reg("MULSUM_K", Src0 * Src1, accum=_add,
                 reference=lambda in0, in1, s0, s1, imm2: in0 * in1)
    masksum = reg("MASKSUM_K", eq(Src0, C0), accum=_add,
                  reference=lambda in0, s0, s1, imm2:
                  (in0 == s0).astype(np.float32))
    return cpack, tpack, mulsum, masksum


def _build(nc, mybir):
    """Emit the SPMD program. Returns (nc, dve_cols, act_cols)."""
    CPACK, TPACK, MULSUM, MASKSUM = _register_ops()
    f32 = mybir.dt.float32
    AF = mybir.ActivationFunctionType
    AL = mybir.AluOpType

    lg = nc.dram_tensor("lg", [B, C, P, F], f32, kind="ExternalInput")
    lb = nc.dram_tensor("lb", [B, P, F], f32, kind="ExternalInput")

    # ---- column bookkeeping ------------------------------------------
    # DVE: ("A0",b,c) | ("T0",b,c) | ("CC",b,c,klo,khi) | ("TP",b,c,klo,khi)
    # ACT: ("A",b,c,k) k=1..14 | ("CS",b,c,k) for SIGN_C
    dve_cols, act_cols = [], []
    sign_c = {}
    for (c, k) in SIGN_C:
        sign_c.setdefault(c, set()).add(k)
    for b in range(B):
        for c in range(C):
            dve_cols.append(("A0", b, c))
            dve_cols.append(("T0", b, c))
            cks = [k for k in range(1, 15) if k not in sign_c.get(c, ())]
            if len(cks) % 2:
                cks.append(cks[-1])
            for i in range(0, len(cks), 2):
                dve_cols.append(("CC", b, c, cks[i], cks[i + 1]))
            tks = list(range(1, 15)) + [14]    # 14 thr -> 7 pairs (pad dup)
            for i in range(0, 14, 2):
                dve_cols.append(("TP", b, c, tks[i], tks[i + 1]))
            for k in range(1, 15):
                act_cols.append(("A", b, c, k))
            for k in sorted(sign_c.get(c, ())):
                act_cols.append(("CS", b, c, k))
    dmap = {it: i for i, it in enumerate(dve_cols)}
    amap = {it: i for i, it in enumerate(act_cols)}

    outV = nc.dram_tensor("outV", [P, len(dve_cols)], f32,
                          kind="ExternalOutput")
    outA = nc.dram_tensor("outA", [P, len(act_cols)], f32,
                          kind="ExternalOutput")

    # ---- const bias APs for ACT --------------------------------------
    bias_vals = {0.0}
    for k in range(1, 15):
        bias_vals.add(-float(TK[k - 1]))
    for v in sorted(bias_vals):
        t = nc.alloc_sbuf_tensor(
            f"cb_{abs(v):.7f}".replace(".", "_") + ("m" if v < 0 else "p"),
            [P, 1], f32)
        nc.gpsimd.memset(t.ap(), v)
        nc.const_aps.aps[(f32, v)] = t.ap()
    nc.all_engine_barrier()

    # ---- sbuf tiles ---------------------------------------------------
    def sb(name, shape, dt=f32):
        return nc.alloc_sbuf_tensor(name, shape, dt).ap()

    lgs = [sb(f"lgs{i}", [P, C * F]) for i in range(2)]   # logits -> e (exp)
    lbs = sb("lbs", [P, F])                               # labels (f32)
    ps = [sb(f"ps{i}", [P, C * F]) for i in range(2)]     # softmax probs
    S = sb("S", [P, F])
    R = sb("R", [P, F])
    rscr = sb("rscr", [P, F])
    m = sb("m", [P, F])                                    # per-class mask
    scr = sb("scr", [P, F])                                # packed-op out
    ascr = sb("ascr", [P, F])                              # ACT singles out
    accV = sb("accV", [P, len(dve_cols)])
    accA = sb("accA", [P, len(act_cols)])

    def pview(buf, c):
        return buf[:, c * F:(c + 1) * F]

    with (
        nc.Block() as block,
        nc.semaphore("dma_sem") as dma_sem,
        nc.semaphore("lg0_sem") as lg0_sem,
        nc.semaphore("lg1_sem") as lg1_sem,
        nc.semaphore("lg2_sem") as lg2_sem,
        nc.semaphore("lg3_sem") as lg3_sem,
        nc.semaphore("lb_sem") as lb_sem,
        nc.semaphore("ae_sem") as ae_sem,      # ACT exp(b) done: b+1
        nc.semaphore("as_sem") as as_sem,      # ACT singles(b) done: b+1
        nc.semaphore("vp_sem") as vp_sem,      # DVE p(b) ready: b+1
        nc.semaphore("vd_sem") as vd_sem,      # DVE packed(b) done: b+1
    ):
        lgc = [lg0_sem, lg1_sem, lg2_sem, lg3_sem]

        @block.sync
        def _(sync):
            for b in range(B):
                if b >= 2:
                    sync.wait_ge(vd_sem, b - 1)
                for c in range(C):
                    sync.dma_start(out=lgs[b % 2][:, c * F:(c + 1) * F],
                                   in_=lg[b, c]).then_inc(lgc[c], 16)
                if b >= 1:
                    sync.wait_ge(vd_sem, b)
                sync.dma_start(out=lbs, in_=lb[b]).then_inc(lb_sem, 16)
            sync.wait_ge(vd_sem, B)
            sync.wait_ge(as_sem, B)
            sync.dma_start(out=outV[:], in_=accV).then_inc(dma_sem, 16)
            sync.dma_start(out=outA[:], in_=accA).then_inc(dma_sem, 16)
            sync.wait_ge(lb_sem, 16 * B)
            sync.wait_ge(dma_sem, 32)

        @block.scalar
        def _(act):
            def exp(b):
                for c in range(C):
                    act.wait_ge(lgc[c], 16 * (b + 1))
                    ins = act.activation(out=pview(lgs[b % 2], c),
                                         in_=pview(lgs[b % 2], c), func=AF.Exp)
                    ins.then_inc(ae_sem, 1)

            def singles(b):
                pb = ps[b % 2]
                ins = None
                for cc in range(C):
                    act.wait_ge(vp_sem, 4 * b + cc + 1)
                    for (fam, bb, c, k) in act_cols:
                        if bb != b or c != cc:
                            continue
                        i0 = amap[(fam, bb, c, k)]
                        ins = act.activation(out=ascr, in_=pview(pb, c),
                                             func=AF.Relu if fam == "A"
                                             else AF.Sign,
                                             bias=-float(TK[k - 1]),
                                             accum_out=accA[:, i0:i0 + 1])
                ins.then_inc(as_sem, 1)

            exp(0)
            exp(1)
            singles(0)
            exp(2)
            singles(1)
            exp(3)
            singles(2)
            singles(3)

        @block.vector
        def _(vec):
            for b in range(B):
                buf = b % 2
                e = lgs[buf]
                pb = ps[buf]
                vec.wait_ge(ae_sem, 4 * b + 2)
                vec.tensor_add(S, pview(e, 0), pview(e, 1))
                vec.wait_ge(ae_sem, 4 * b + 3)
                vec.tensor_add(S, S, pview(e, 2))
                vec.wait_ge(ae_sem, 4 * b + 4)
                vec.tensor_add(S, S, pview(e, 3))
                vec.reciprocal_approx_accurate(out=R, in_=S, scratch=rscr)
                if b >= 2:
                    vec.wait_ge(as_sem, b - 1)
                for c in range(C):
                    ao = accV[:, dmap[("A0", b, c)]:dmap[("A0", b, c)] + 1]
                    vec._custom_dve(MULSUM, out=pview(pb, c),
                                    in0=pview(e, c), in1=R,
                                    accum_out=ao).then_inc(vp_sem, 1)
                vec.wait_ge(lb_sem, 16 * (b + 1))
                for c in range(C):
                    ao = accV[:, dmap[("T0", b, c)]:dmap[("T0", b, c)] + 1]
                    vec._custom_dve(MASKSUM, out=m, in0=lbs,
                                    s0=float(c), accum_out=ao)
                    for it in dve_cols:
                        if it[0] == "TP" and it[1] == b and it[2] == c:
                            _, _, _, klo, khi = it
                            ao2 = accV[:, dmap[it]:dmap[it] + 1]
                            vec._custom_dve(
                                TPACK, out=scr, in0=pview(pb, c), in1=m,
                                s0=float(TK[klo - 1]), s1=float(TK[khi - 1]),
                                imm2=PK, accum_out=ao2)
                    for it in dve_cols:
                        if it[0] == "CC" and it[1] == b and it[2] == c:
                            _, _, _, klo, khi = it
                            ao2 = accV[:, dmap[it]:dmap[it] + 1]
                            ins = vec._custom_dve(
                                CPACK, out=scr, in0=pview(pb, c),
                                s0=float(TK[klo - 1]), s1=float(TK[khi - 1]),
                                imm2=PK, accum_out=ao2)
                ins.then_inc(vd_sem, 1)

    return nc, dve_cols, act_cols, dmap, amap


def _decode(dve_cols, act_cols, results):
    """Sum per-core [128, n] accumulators and decode into the cumulative
    family arrays Cf[b,c,k], Af[b,c,k], Tf[b,c,k] (k = 0..15)."""
    NV = len(dve_cols)
    totV = np.zeros(NV, np.float64)
    totA = np.zeros(len(act_cols), np.float64)
    # packed columns must be decoded per partition-row per core (fields are
    # only guaranteed <= 2048 per row), so split lo/hi before summing.
    lo_acc = np.zeros(NV, np.float64)
    hi_acc = np.zeros(NV, np.float64)
    for r in results:
        v = r["outV"].astype(np.float64)        # [128, NV]
        hi = np.floor(v / PK)
        lo = v - hi * PK
        lo_acc += lo.sum(0)
        hi_acc += hi.sum(0)
        totV += v.sum(0)
        totA += r["outA"].astype(np.float64).sum(0)

    Cf = np.zeros((B, C, 16))
    Af = np.zeros((B, C, 16))
    Tf = np.zeros((B, C, 16))
    Cf[:, :, 0] = SP_FULL
    n_cores = len(results)
    for i, it in enumerate(dve_cols):
        fam = it[0]
        if fam == "A0":
            Af[it[1], it[2], 0] = totV[i]
        elif fam == "T0":
            Tf[it[1], it[2], 0] = totV[i]
        elif fam == "CC":
            _, b, c, klo, khi = it
            Cf[b, c, klo] = lo_acc[i]
            Cf[b, c, khi] = hi_acc[i]
        else:  # TP
            _, b, c, klo, khi = it
            Tf[b, c, klo] = lo_acc[i]
            Tf[b, c, khi] = hi_acc[i]
    for i, it in enumerate(act_cols):
        fam, b, c, k = it
        if fam == "A":
            Af[b, c, k] = totA[i]
        else:  # CS: sign-encoded count
            Cf[b, c, k] = (totA[i] + SP_FULL) / 2.0
    return Cf, Af, Tf


def _finalize(Cf, Af, Tf):
    tk = np.zeros(16)
    tk[1:16] = TK.astype(np.float64)
    cnt = Cf[:, :, :15] - Cf[:, :, 1:16]
    S = Af[:, :, :15] + tk[:15] * Cf[:, :, :15]
    Sb = np.zeros((B, C, 15))
    Sb[:, :, :14] = S[:, :, :14] - S[:, :, 1:15]
    Sb[:, :, 14] = S[:, :, 14]
    tcb = Tf[:, :, :15] - Tf[:, :, 1:16]

    valid = cnt > 0.5
    denom = np.where(valid, cnt, 1.0)
    mean_p = Sb / denom
    mean_t = tcb / denom
    diff = np.where(valid, np.abs(mean_p - mean_t), 0.0)
    n_valid = np.maximum(valid.sum(-1), 1)
    ace = diff.sum(-1) / n_valid
    non_empty = (Tf[:, :, 0] > 0.5).astype(np.float64)
    return np.float32((ace * non_empty).mean())


def kernel(logits, labels):
    import concourse.bass as bass
    from concourse import mybir
    from concourse.bass_utils import run_bass_kernel_spmd

    nc = bass.Bass()
    nc, dve_cols, act_cols, dmap, amap = _build(nc, mybir)
    mybir.codegen_inst_isa_subclasses(nc)   # encode custom-DVE ISA bytes

    lgf = np.ascontiguousarray(np.asarray(logits).reshape(B, C, SP_FULL),
                               np.float32)
    lbl = np.asarray(labels).reshape(B, SP_FULL).astype(np.float32)

    in_maps = []
    for i in range(NCORES):
        sl = slice(i * SP, (i + 1) * SP)
        in_maps.append({
            "lg": np.ascontiguousarray(lgf[:, :, sl]).reshape(B, C, P, F),
            "lb": np.ascontiguousarray(lbl[:, sl]).reshape(B, P, F),
        })
    trace = bool(int(os.environ.get("KERNEL_TRACE", "0")))
    tmpdir = os.environ.get("KERNEL_TMPDIR") or None
    res = run_bass_kernel_spmd(nc, in_maps, list(range(NCORES)), trace=trace,
                               tmpdir=tmpdir)
    Cf, Af, Tf = _decode(dve_cols, act_cols, res.results)
    out = _finalize(Cf, Af, Tf)
    kernel._last = res
    return out


# revision 8
# speedup vs baseline: 1.7678x; 1.0175x over previous
"""HL1 ACE loss kernel for Trainium2, 8-core data-parallel over spatial.

Per core (per batch b): softmax over C=4 on the spatial shard, then the
three per-(b,c,bin) histogram families via cumulative thresholds:
  C_k = #{p_c >= t_k}          counts
  A_k = sum relu(p_c - t_k)    prob-mass above t_k  (=> per-bin sum_p)
  T_k = #{lab==c & p_c >= t_k} target counts
Custom DVE micro-ops pack TWO counts per pass into one f32 accumulator
(lo + 4096*hi; both fields <= 2048 so the sum stays integer-exact under
2^24), and fuse p-materialization with A_0 (MULSUM) and mask-build with
T_0 (MASKSUM). ACT carries exp, the relu (A) singles and a few sign (C)
singles. Host decodes the tiny [128, ncols] accumulators.
"""
import sys
sys.path.insert(0, "/opt/trn_rl_repo")
import os
import numpy as np

B, C = 4, 4
NBINS = 15
NCORES = 8
SP_FULL = 128 * 128 * 128          # spatial per (b,c), full problem
SP = SP_FULL // NCORES             # spatial per core = 262144
P, F = 128, SP // 128              # sbuf tile geometry 128 x 2048

EPS32 = np.float32(np.finfo(np.float32).eps)
BOUNDS = np.linspace(np.float32(0.0), np.float32(1.0) + EPS32, NBINS + 1,
                     dtype=np.float32)
TK = BOUNDS[1:]                    # t_1..t_15 (t_15 = 1+eps, never used)

PK = 4096.0                        # packing field multiplier

# ACT takes these C-thresholds as Sign singles; the rest pair up on DVE.
SIGN_C = [(0, 14), (1, 14), (2, 14), (3, 14),
          (0, 13), (1, 13), (2, 13), (3, 13)]


# ---- custom DVE op registration ------------------------------------------
def _register_ops():
    import concourse.dve_ops as dops
    from concourse.dve_spec import (Spec, Src0, Src1, C0, C1, C2, relu, eq,
                                    lower, _has_src1)
    from concourse.dve_uop import DveOpSpec
    from operator import add as _add

    def reg(name, body, accum=None, reference=None):
        for o in dops.OPS:
            if o.name == name:
                return o
        row = dops._CUSTOM_DVE_ROW_BASE + len(dops.OPS)
        spec = Spec(body=body, accum=accum, reference=reference)
        sha = {}
        for ver in ("v3", "v4"):
            u = lower(spec, ver=ver)
            sha[ver] = DveOpSpec(name=name, opcode=row, uops=u,
                                 rd1_en=_has_src1(spec)).sha(ver)
        op = dops.DveOp(name, spec, subdim=False, uops_sha=sha)
        dops.OPS.append(op)
        dops._SUB_OPCODE_FOR_NAME[name] = row
        dops.CUSTOM_DVE_SPECS[name] = spec
        return op

    cpack = reg("CPACK_K", (Src0 >= C0) + C2 * (Src0 >= C1), accum=_add,
                reference=lambda in0, s0, s1, imm2:
                (in0 >= s0) + imm2 * (in0 >= s1))
    tpack = reg("TPACK_K", ((Src0 >= C0) + C2 * (Src0 >= C1)) * Src1,
                accum=_add,
                reference=lambda in0, in1, s0, s1, imm2:
                ((in0 >= s0) + imm2 * (in0 >= s1)) * in1)
    mulsum = reg("MULSUM_K", Src0 * Src1, accum=_add,
                 reference=lambda in0, in1, s0, s1, imm2: in0 * in1)
    masksum = reg("MASKSUM_K", eq(Src0, C0), accum=_add,
                  reference=lambda in0, s0, s1, imm2:
                  (in0 == s0).astype(np.float32))
    return cpack, tpack, mulsum, masksum


def _build(nc, mybir):
    """Emit the SPMD program. Returns (nc, dve_cols, act_cols)."""
    CPACK, TPACK, MULSUM, MASKSUM = _register_ops()
    f32 = mybir.dt.float32
    AF = mybir.ActivationFunctionType
    AL = mybir.AluOpType

    lg = nc.dram_tensor("lg", [B, C, P, F], f32, kind="ExternalInput")
    lb = nc.dram_tensor("lb", [B, P, F], f32, kind="ExternalInput")

    # ---- column bookkeeping ------------------------------------------
    # DVE: ("A0",b,c) | ("T0",b,c) | ("CC",b,c,klo,khi) | ("TP",b,c,klo,khi)
    # ACT: ("A",b,c,k) k=1..14 | ("CS",b,c,k) for SIGN_C
    dve_cols, act_cols = [], []
    sign_c = {}
    for (c, k) in SIGN_C:
        sign_c.setdefault(c, set()).add(k)
    for b in range(B):
        for c in range(C):
            dve_cols.append(("A0", b, c))
            dve_cols.append(("T0", b, c))
            cks = [k for k in range(1, 15) if k not in sign_c.get(c, ())]
            if len(cks) % 2:
                cks.append(cks[-1])
            for i in range(0, len(cks), 2):
                dve_cols.append(("CC", b, c, cks[i], cks[i + 1]))
            tks = list(range(1, 15)) + [14]    # 14 thr -> 7 pairs (pad dup)
            for i in range(0, 14, 2):
                dve_cols.append(("TP", b, c, tks[i], tks[i + 1]))
            for k in range(1, 15):
                act_cols.append(("A", b, c, k))
            for k in sorted(sign_c.get(c, ())):
                act_cols.append(("CS", b, c, k))
    dmap = {it: i for i, it in enumerate(dve_cols)}
    amap = {it: i for i, it in enumerate(act_cols)}

    outV = nc.dram_tensor("outV", [P, len(dve_cols)], f32,
                          kind="ExternalOutput")
    outA = nc.dram_tensor("outA", [P, len(act_cols)], f32,
                          kind="ExternalOutput")

    # ---- const bias APs for ACT --------------------------------------
    bias_vals = {0.0}
    for k in range(1, 15):
        bias_vals.add(-float(TK[k - 1]))
    for v in sorted(bias_vals):
        t = nc.alloc_sbuf_tensor(
            f"cb_{abs(v):.7f}".replace(".", "_") + ("m" if v < 0 else "p"),
            [P, 1], f32)
        nc.gpsimd.memset(t.ap(), v)
        nc.const_aps.aps[(f32, v)] = t.ap()
    nc.all_engine_barrier()

    # ---- sbuf tiles ---------------------------------------------------
    def sb(name, shape, dt=f32):
        return nc.alloc_sbuf_tensor(name, shape, dt).ap()

    lgs = [sb(f"lgs{i}", [P, C * F]) for i in range(2)]   # logits -> e (exp)
    lbs = sb("lbs", [P, F])                               # labels (f32)
    ps = [sb(f"ps{i}", [P, C * F]) for i in range(2)]     # softmax probs
    S = sb("S", [P, F])
    R = sb("R", [P, F])
    rscr = sb("rscr", [P, F])
    m = sb("m", [P, F])                                    # per-class mask
    scr = sb("scr", [P, F])                                # packed-op out
    ascr = sb("ascr", [P, F])                              # ACT singles out
    accV = sb("accV", [P, len(dve_cols)])
    accA = sb("accA", [P, len(act_cols)])

    def pview(buf, c):
        return buf[:, c * F:(c + 1) * F]

    with (
        nc.Block() as block,
        nc.semaphore("dma_sem") as dma_sem,
        nc.semaphore("lg0_sem") as lg0_sem,
        nc.semaphore("lg1_sem") as lg1_sem,
        nc.semaphore("lg2_sem") as lg2_sem,
        nc.semaphore("lg3_sem") as lg3_sem,
        nc.semaphore("lb_sem") as lb_sem,
        nc.semaphore("ae_sem") as ae_sem,      # ACT exp(b) done: b+1
        nc.semaphore("as_sem") as as_sem,      # ACT singles(b) done: b+1
        nc.semaphore("vp_sem") as vp_sem,      # DVE p(b) ready: b+1
        nc.semaphore("vd_sem") as vd_sem,      # DVE packed(b) done: b+1
    ):
        lgc = [lg0_sem, lg1_sem, lg2_sem, lg3_sem]

        @block.sync
        def _(sync):
            for b in range(B):
                if b >= 2:
                    sync.wait_ge(vd_sem, b - 1)
                for c in range(C):
                    sync.dma_start(out=lgs[b % 2][:, c * F:(c + 1) * F],
                                   in_=lg[b, c]).then_inc(lgc[c], 16)
                if b >= 1:
                    sync.wait_ge(vd_sem, b)
                sync.dma_start(out=lbs, in_=lb[b]).then_inc(lb_sem, 16)
            sync.wait_ge(vd_sem, B)
            sync.wait_ge(as_sem, B)
            sync.dma_start(out=outV[:], in_=accV).then_inc(dma_sem, 16)
            sync.dma_start(out=outA[:], in_=accA).then_inc(dma_sem, 16)
            sync.wait_ge(lb_sem, 16 * B)
            sync.wait_ge(dma_sem, 32)

        @block.scalar
        def _(act):
            def exp(b):
                for c in range(C):
                    act.wait_ge(lgc[c], 16 * (b + 1))
                    ins = act.activation(out=pview(lgs[b % 2], c),
                                         in_=pview(lgs[b % 2], c), func=AF.Exp)
                    ins.then_inc(ae_sem, 1)

            def singles(b):
                pb = ps[b % 2]
                ins = None
                for cc in range(C):
                    act.wait_ge(vp_sem, 4 * b + cc + 1)
                    for (fam, bb, c, k) in act_cols:
                        if bb != b or c != cc:
                            continue
                        i0 = amap[(fam, bb, c, k)]
                        ins = act.activation(out=ascr, in_=pview(pb, c),
                                             func=AF.Relu if fam == "A"
                                             else AF.Sign,
                                             bias=-float(TK[k - 1]),
                                             accum_out=accA[:, i0:i0 + 1])
                ins.then_inc(as_sem, 1)

            exp(0)
            exp(1)
            singles(0)
            exp(2)
            singles(1)
            exp(3)
            singles(2)
            singles(3)

        @block.vector
        def _(vec):
            for b in range(B):
                buf = b % 2
                e = lgs[buf]
                pb = ps[buf]
                vec.wait_ge(ae_sem, 4 * b + 2)
                vec.tensor_add(S, pview(e, 0), pview(e, 1))
                vec.wait_ge(ae_sem, 4 * b + 3)
                vec.tensor_add(S, S, pview(e, 2))
                vec.wait_ge(ae_sem, 4 * b + 4)
                vec.tensor_add(S, S, pview(e, 3))
                vec.reciprocal_approx_fast(out=R, in_=S)
                if b >= 2:
                    vec.wait_ge(as_sem, b - 1)
                for c in range(C):
                    ao = accV[:, dmap[("A0", b, c)]:dmap[("A0", b, c)] + 1]
                    vec._custom_dve(MULSUM, out=pview(pb, c),
                                    in0=pview(e, c), in1=R,
                                    accum_out=ao).then_inc(vp_sem, 1)
                vec.wait_ge(lb_sem, 16 * (b + 1))
                for c in range(C):
                    ao = accV[:, dmap[("T0", b, c)]:dmap[("T0", b, c)] + 1]
                    vec._custom_dve(MASKSUM, out=m, in0=lbs,
                                    s0=float(c), accum_out=ao)
                    for it in dve_cols:
                        if it[0] == "TP" and it[1] == b and it[2] == c:
                            _, _, _, klo, khi = it
                            ao2 = accV[:, dmap[it]:dmap[it] + 1]
                            vec._custom_dve(
                                TPACK, out=scr, in0=pview(pb, c), in1=m,
                                s0=float(TK[klo - 1]), s1=float(TK[khi - 1]),
                                imm2=PK, accum_out=ao2)
                    for it in dve_cols:
                        if it[0] == "CC" and it[1] == b and it[2] == c:
                            _, _, _, klo, khi = it
                            ao2 = accV[:, dmap[it]:dmap[it] + 1]
                            ins = vec._custom_dve(
                                CPACK, out=scr, in0=pview(pb, c),
                                s0=float(TK[klo - 1]), s1=float(TK[khi - 1]),
                                imm2=PK, accum_out=ao2)
                ins.then_inc(vd_sem, 1)

    return nc, dve_cols, act_cols, dmap, amap


def _decode(dve_cols, act_cols, results):
    """Sum per-core [128, n] accumulators and decode into the cumulative
    family arrays Cf[b,c,k], Af[b,c,k], Tf[b,c,k] (k = 0..15)."""
    NV = len(dve_cols)
    totV = np.zeros(NV, np.float64)
    totA = np.zeros(len(act_cols), np.float64)
    # packed columns must be decoded per partition-row per core (fields are
    # only guaranteed <= 2048 per row), so split lo/hi before summing.
    lo_acc = np.zeros(NV, np.float64)
    hi_acc = np.zeros(NV, np.float64)
    for r in results:
        v = r["outV"].astype(np.float64)        # [128, NV]
        hi = np.floor(v / PK)
        lo = v - hi * PK
        lo_acc += lo.sum(0)
        hi_acc += hi.sum(0)
        totV += v.sum(0)
        totA += r["outA"].astype(np.float64).sum(0)

    Cf = np.zeros((B, C, 16))
    Af = np.zeros((B, C, 16))
    Tf = np.zeros((B, C, 16))
    Cf[:, :, 0] = SP_FULL
    n_cores = len(results)
    for i, it in enumerate(dve_cols):
        fam = it[0]
        if fam == "A0":
            Af[it[1], it[2], 0] = totV[i]
        elif fam == "T0":
            Tf[it[1], it[2], 0] = totV[i]
        elif fam == "CC":
            _, b, c, klo, khi = it
            Cf[b, c, klo] = lo_acc[i]
            Cf[b, c, khi] = hi_acc[i]
        else:  # TP
            _, b, c, klo, khi = it
            Tf[b, c, klo] = lo_acc[i]
            Tf[b, c, khi] = hi_acc[i]
    for i, it in enumerate(act_cols):
        fam, b, c, k = it
        if fam == "A":
            Af[b, c, k] = totA[i]
        else:  # CS: sign-encoded count
            Cf[b, c, k] = (totA[i] + SP_FULL) / 2.0
    return Cf, Af, Tf


def _finalize(Cf, Af, Tf):
    tk = np.zeros(16)
    tk[1:16] = TK.astype(np.float64)
    cnt = Cf[:, :, :15] - Cf[:, :, 1:16]
    S = Af[:, :, :15] + tk[:15] * Cf[:, :, :15]
    Sb = np.zeros((B, C, 15))
    Sb[:, :, :14] = S[:, :, :14] - S[:, :, 1:15]
    Sb[:, :, 14] = S[:, :, 14]
    tcb = Tf[:, :, :15] - Tf[:, :, 1:16]

    valid = cnt > 0.5
    denom = np.where(valid, cnt, 1.0)
    mean_p = Sb / denom
    mean_t = tcb / denom
    diff = np.where(valid, np.abs(mean_p - mean_t), 0.0)
    n_valid = np.maximum(valid.sum(-1), 1)
    ace = diff.sum(-1) / n_valid
    non_empty = (Tf[:, :, 0] > 0.5).astype(np.float64)
    return np.float32((ace * non_empty).mean())


def kernel(logits, labels):
    import concourse.bass as bass
    from concourse import mybir
    from concourse.bass_utils import run_bass_kernel_spmd

    nc = bass.Bass()
    nc, dve_cols, act_cols, dmap, amap = _build(nc, mybir)
    mybir.codegen_inst_isa_subclasses(nc)   # encode custom-DVE ISA bytes

    lgf = np.ascontiguousarray(np.asarray(logits).reshape(B, C, SP_FULL),
                               np.float32)
    lbl = np.asarray(labels).reshape(B, SP_FULL).astype(np.float32)

    in_maps = []
    for i in range(NCORES):
        sl = slice(i * SP, (i + 1) * SP)
        in_maps.append({
            "lg": np.ascontiguousarray(lgf[:, :, sl]).reshape(B, C, P, F),
            "lb": np.ascontiguousarray(lbl[:, sl]).reshape(B, P, F),
        })
    trace = bool(int(os.environ.get("KERNEL_TRACE", "0")))
    tmpdir = os.environ.get("KERNEL_TMPDIR") or None
    res = run_bass_kernel_spmd(nc, in_maps, list(range(NCORES)), trace=trace,
                               tmpdir=tmpdir)
    Cf, Af, Tf = _decode(dve_cols, act_cols, res.results)
    out = _finalize(Cf, Af, Tf)
    kernel._last = res
    return out
